# revision 26
# baseline (speedup 1.0000x reference)
"""Trainium2 Bass kernel for nn_Experts (grouped MoE expert MLP).

Computes, for each of 8 experts e:
    h   = x_e @ w0_e.T          # [2048,1024] @ [1024,4096] -> [2048,4096]
    g   = gelu_exact(h)
    out = g @ w3_e.T            # [2048,4096] @ [4096,1024] -> [2048,1024]
then masks unpopular experts with zero gating activity (output_tensor).

Sharding: expert-parallel, 1 expert per NeuronCore across 8 cores (SPMD).

Compute strategy: error-compensated fp8 (e4m3) matmuls in DoubleRow perf
mode (K=256 per instruction, 0.5 PE cycles per output element -- 4x bf16
per fp8 pass). Plain fp8 quantization error (~5%) exceeds the accuracy
gate, so each GEMM runs TWO fp8 passes whose second ("residual") pass
cancels the first-order quantization error of BOTH operands:

    A1 = q(X/g),  A2 = q(X - A1)          (g = 1+c, c = 0.125)
    B1 = q(W),    B2 = q(W(1 + 1/c) - B1/c)
    X @ W  ~  A1@B1 + A2@B2               (error ~1e-2 absmax ratio)

The (1+c) factors cancel algebraically, so no rescale is needed beyond
the static weight scale S=64 folded into the GELU input scale / output
copy scale. Two fp8 passes at 4x = net 2x over the bf16 baseline.

x/w0/w3 terms are host-prepped; the GEMM2 stationary operand (gelu
output) is produced on device: ACT computes g_bf = gelu(h/S) (bf16) and
g1 = q(g_bf/g) (fp8 copy with scale), DVE computes gr = q(g_bf - g1).
"""

import numpy as np
import ml_dtypes

T = 2048      # tokens (capacity) per expert
D = 1024      # hidden
F = 4096      # ffn
P = 128       # partitions
TBS = 512     # token block
NTBS = T // TBS
KC1 = D // 256   # GEMM1 k-chunks (K=256 per DoubleRow matmul)
KC2 = F // 256   # GEMM2 k-chunks
G = 8            # w0 f-groups (DMA granularity)
FW = F // G      # f columns per group
NPAIR = F // 256  # gelu-tile pairs per block (16)
NTS = TBS // P    # t-subtiles per block (GEMM2 output partitions)
NUM_LOCAL = 4
N_CORES = 8

C = 0.125        # residual compensation ratio
GAM = 1.0 + C
S = 64.0         # static weight scale (keeps fp8(w) out of subnormals)

_cache = {}


def _build_nc(
    g_extra=1,
    h_bufs=3,
    o_ps_bufs=5,
    o_sb_bufs=3,
    x_bufs=2,
    gbf_bufs=4,
    warmup_mms=14,
):
    import sys
    if "/opt/trn_rl_repo" not in sys.path:
        sys.path.insert(0, "/opt/trn_rl_repo")
    import concourse.tile as tile
    import concourse.mybir as mybir
    from concourse import bacc

    bf16 = mybir.dt.bfloat16
    f32 = mybir.dt.float32
    fp8 = mybir.dt.float8e4
    AFT = mybir.ActivationFunctionType
    DR = mybir.MatmulPerfMode.DoubleRow
    SUB = mybir.AluOpType.subtract

    nc = bacc.Bacc(
        "TRN2",
        target_bir_lowering=False,
        debug=False,
        enable_asserts=True,
        num_devices=N_CORES,
    )

    # DRAM layouts: partition = contraction%128, plane dim of 2 for DoubleRow
    # (contraction index k = 256*kc + 128*i + p), grouped for long DMA runs.
    xA1 = nc.dram_tensor("xA1", [P, NTBS, KC1, 2, TBS], fp8, kind="ExternalInput").ap()
    xA2 = nc.dram_tensor("xA2", [P, NTBS, KC1, 2, TBS], fp8, kind="ExternalInput").ap()
    w0B1 = nc.dram_tensor("w0B1", [P, G, KC1, 2, FW], fp8, kind="ExternalInput").ap()
    w0B2 = nc.dram_tensor("w0B2", [P, G, KC1, 2, FW], fp8, kind="ExternalInput").ap()
    w3C1 = nc.dram_tensor("w3C1", [P, KC2, 2, D], fp8, kind="ExternalInput").ap()
    w3C2 = nc.dram_tensor("w3C2", [P, KC2, 2, D], fp8, kind="ExternalInput").ap()
    out = nc.dram_tensor("out", [T, D], f32, kind="ExternalOutput").ap()

    with tile.TileContext(nc) as tc:
        with (
            tc.tile_pool(name="weights", bufs=1) as wpool,
            tc.tile_pool(name="xin", bufs=x_bufs) as xpool,
            tc.tile_pool(name="gbf", bufs=gbf_bufs) as gbfpool,
            tc.tile_pool(name="g1p", bufs=NPAIR + g_extra) as g1pool,
            tc.tile_pool(name="grp", bufs=NPAIR + g_extra) as grpool,
            tc.tile_pool(name="ostage", bufs=o_sb_bufs) as opool,
            tc.tile_pool(name="hps", bufs=h_bufs, space="PSUM") as hpsum,
            tc.tile_pool(name="ops", bufs=o_ps_bufs, space="PSUM") as opsum,
        ):
            w0_sb = [wpool.tile([P, G, KC1, 2, FW], fp8, name=f"w0_sb{i}", tag=f"w0_sb{i}")
                     for i in range(2)]
            w3_sb = [wpool.tile([P, KC2, 2, D], fp8, name=f"w3_sb{i}", tag=f"w3_sb{i}")
                     for i in range(2)]

            x_tiles = {}

            def load_x(tb):
                xt = [xpool.tile([P, KC1, 2, TBS], fp8, name=f"x{i}_{tb}", tag=f"x{i}")
                      for i in range(2)]
                nc.sync.dma_start(xt[0][:], xA1[:, tb])
                nc.sync.dma_start(xt[1][:], xA2[:, tb])
                x_tiles[tb] = xt

            if warmup_mms:
                # ride out the PE cold-clock window on scratch matmuls while
                # the first DMAs are in flight
                with tc.tile_pool(name="warm", bufs=1) as warmpool:
                    wsrc = warmpool.tile([P, TBS], bf16, name="wsrc", tag="wsrc")
                    wps = opsum.tile([P, TBS], f32, name="wps", tag="o_ps")
                    nc.gpsimd.memset(wsrc[:], 0.0)
                    for i in range(warmup_mms):
                        nc.tensor.matmul(wps[:], wsrc[:, :P], wsrc[:],
                                         start=(i == 0), stop=(i == warmup_mms - 1))

            # critical prefix: w0 group 0, then x[tb0] (first matmul needs
            # both), then remaining w0 groups in consumption order, then w3
            # by k-chunk (phase B consumes kc2-outer), then the rest of x.
            nc.sync.dma_start(w0_sb[0][:, 0], w0B1[:, 0])
            nc.sync.dma_start(w0_sb[1][:, 0], w0B2[:, 0])
            load_x(0)
            w3_head = {3: 0, 6: 1}  # w3 chunks slipped into w0 stream slack
            for g in range(1, G):
                nc.sync.dma_start(w0_sb[0][:, g], w0B1[:, g])
                nc.sync.dma_start(w0_sb[1][:, g], w0B2[:, g])
                if g in w3_head:
                    kc2 = w3_head[g]
                    nc.sync.dma_start(w3_sb[0][:, kc2], w3C1[:, kc2])
                    nc.sync.dma_start(w3_sb[1][:, kc2], w3C2[:, kc2])
            for kc2 in range(len(w3_head), KC2):
                nc.sync.dma_start(w3_sb[0][:, kc2], w3C1[:, kc2])
                nc.sync.dma_start(w3_sb[1][:, kc2], w3C2[:, kc2])
            for tb in range(1, NTBS):
                load_x(tb)

            for tb in range(NTBS):
                xt = x_tiles.pop(tb)

                # phase A: GEMM1 (2-term fp8 DR) + GELU for all 32 f-tiles;
                # fp8 g1/gr pair tiles [P, 2, TBS] feed phase B as stationary.
                g1_t, gr_t = [], []
                for pair in range(NPAIR):
                    g1 = g1pool.tile([P, 2, TBS], fp8, name=f"g1_{tb}_{pair}", tag="g1")
                    gr = grpool.tile([P, 2, TBS], fp8, name=f"gr_{tb}_{pair}", tag="gr")
                    g1_t.append(g1)
                    gr_t.append(gr)
                    for i in range(2):
                        fc = 2 * pair + i
                        g, j = fc // (FW // P), fc % (FW // P)
                        h_ps = hpsum.tile([P, TBS], f32, name=f"h_{tb}_{fc}", tag="h_ps")
                        for term in range(2):
                            for kc in range(KC1):
                                nc.tensor.matmul(
                                    h_ps[:],
                                    w0_sb[term][:, g, kc, :, j * P:(j + 1) * P],
                                    xt[term][:, kc],
                                    start=(term == 0 and kc == 0),
                                    stop=(term == 1 and kc == KC1 - 1),
                                    perf_mode=DR,
                                )
                        g_bf = gbfpool.tile([P, TBS], bf16, name=f"gb_{tb}_{fc}", tag="g_bf")
                        # gelu alone on ACT (gates h_ps release; must outpace
                        # PE's 853ns/f-tile); g1/gr on DVE (delay-tolerant)
                        nc.scalar.activation(g_bf[:], h_ps[:], AFT.Gelu, scale=1.0 / S)
                        nc.vector.tensor_scalar_mul(g1[:, i], g_bf[:], 1.0 / GAM)
                        nc.vector.tensor_tensor(gr[:, i], g_bf[:], g1[:, i], SUB)

                # phase B: GEMM2 over the 8 [128t x 512d] psum out tiles in
                # two kc2-outer groups of 5 then 3: kc2-outer ordering lets
                # w3 stream in k-chunk order (block 0) and tolerates DVE
                # g1/gr lag; the bigger first group keeps PE fed while w3
                # still streams, the smaller second group shrinks the
                # post-delivery tail. The final group of the last block runs
                # tile-sequential so copies/DMA-out pipeline behind the
                # matmuls instead of serializing after the last one.
                tiles = [(ts, dc) for ts in range(NTS) for dc in range(2)]
                # block 0: bigger first group = more PE work per streamed-in
                # w3 k-chunk (higher tolerance to the w3 DMA stream)
                split = 5 if tb == 0 else 4
                groups = [tiles[:split], tiles[split:]]

                def emit_mm(o_t, ts, dc, kc2):
                    for term in range(2):
                        gt = (g1_t, gr_t)[term][kc2]
                        nc.tensor.matmul(
                            o_t[:],
                            gt[:, :, ts * P:(ts + 1) * P],
                            w3_sb[term][:, kc2, :, dc * TBS:(dc + 1) * TBS],
                            start=(kc2 == 0 and term == 0),
                            stop=(kc2 == KC2 - 1 and term == 1),
                            perf_mode=DR,
                        )

                def emit_copy(o_t, ts, dc, splits=1, eng=None):
                    # group1 copies go to DVE (drained by end of phase B) so
                    # ACT's queue holds only gelus at the next phase-A start
                    # — gelu latency gates h_ps reuse
                    o_sb = opool.tile([P, TBS], f32, name=f"os_{tb}_{ts}_{dc}",
                                      tag="o_sb")
                    w = TBS // splits
                    for s in range(splits):
                        if eng is nc.scalar:
                            nc.scalar.mul(o_sb[:, s * w:(s + 1) * w],
                                          o_t[:, s * w:(s + 1) * w], 1.0 / S)
                        else:
                            nc.vector.tensor_scalar_mul(
                                o_sb[:, s * w:(s + 1) * w],
                                o_t[:, s * w:(s + 1) * w], 1.0 / S)
                        nc.sync.dma_start(
                            out[tb * TBS + ts * P: tb * TBS + (ts + 1) * P,
                                dc * TBS + s * w: dc * TBS + (s + 1) * w],
                            o_sb[:, s * w:(s + 1) * w],
                        )

                for gi, group in enumerate(groups):
                    o_ps = {td: opsum.tile([P, TBS], f32,
                                           name=f"o_{tb}_{td[0]}_{td[1]}", tag="o_ps")
                            for td in group}
                    if tb == NTBS - 1 and gi == 1:
                        for i, (ts, dc) in enumerate(group):
                            last = i == len(group) - 1
                            for kc2 in range(KC2):
                                emit_mm(o_ps[(ts, dc)], ts, dc, kc2)
                            emit_copy(o_ps[(ts, dc)], ts, dc,
                                      splits=2 if last else 1,
                                      eng=nc.scalar if last else None)
                    else:
                        for kc2 in range(KC2):
                            for ts, dc in group:
                                emit_mm(o_ps[(ts, dc)], ts, dc, kc2)
                        for ts, dc in group:
                            emit_copy(o_ps[(ts, dc)], ts, dc, eng=nc.scalar)

    nc.compile()
    return nc


def _get_nc():
    if "nc" not in _cache:
        _cache["nc"] = _build_nc()
    return _cache["nc"]


def _make_cached_fn(nc):
    """Build a reusable jitted 8-core executable around bass2jax's bass_exec
    primitive (the same lowering run_bass_kernel_spmd uses under axon), so
    repeat kernel() calls skip retrace/relower."""
    import jax
    import numpy as np
    from jax.sharding import Mesh, PartitionSpec
    try:
        from jax.experimental.shard_map import shard_map
    except ImportError:
        from jax.shard_map import shard_map
    import concourse.mybir as mybir
    from concourse.bass2jax import (_bass_exec_p, install_neuronx_cc_hook,
                                    partition_id_tensor)

    install_neuronx_cc_hook()
    partition_name = nc.partition_id_tensor.name if nc.partition_id_tensor else None
    in_names, out_names, out_avals, zero_shapes = [], [], [], []
    for alloc in nc.m.functions[0].allocations:
        if not isinstance(alloc, mybir.MemoryLocationSet):
            continue
        name = alloc.memorylocations[0].name
        if alloc.kind == "ExternalInput":
            if name != partition_name:
                in_names.append(name)
        elif alloc.kind == "ExternalOutput":
            out_names.append(name)
            shape = tuple(alloc.tensor_shape)
            dtype = mybir.dt.np(alloc.dtype)
            out_avals.append(jax.core.ShapedArray(shape, dtype))
            zero_shapes.append((shape, dtype))
    n_params = len(in_names)
    all_in_names = list(in_names) + list(out_names)
    if partition_name is not None:
        all_in_names.append(partition_name)

    def _body(*args):
        ins = list(args[:n_params])
        outs = list(args[n_params:])
        extra = [partition_id_tensor()] if partition_name is not None else []
        return tuple(_bass_exec_p.bind(
            *ins, *outs, *extra,
            out_avals=tuple(out_avals),
            in_names=tuple(all_in_names),
            out_names=tuple(out_names),
            lowering_input_output_aliases=(),
            sim_require_finite=True,
            sim_require_nnan=True,
            nc=nc,
        ))

    devices = jax.devices()[:N_CORES]
    mesh = Mesh(np.asarray(devices), ("core",))
    fn = jax.jit(
        shard_map(_body, mesh=mesh,
                  in_specs=(PartitionSpec("core"),) * (n_params + len(out_names)),
                  out_specs=(PartitionSpec("core"),) * len(out_names),
                  check_rep=False),
        keep_unused=True)

    def run(in_maps):
        concat_in = [np.concatenate([np.asarray(m[n]) for m in in_maps], axis=0)
                     for n in in_names]
        concat_zeros = [np.zeros((N_CORES * s[0], *s[1:]), dt)
                        for s, dt in zero_shapes]
        outs = fn(*concat_in, *concat_zeros)
        return [
            {name: np.asarray(outs[i]).reshape(N_CORES, *out_avals[i].shape)[c]
             for i, name in enumerate(out_names)}
            for c in range(N_CORES)
        ]

    return run


def _prep_expert(x, w0, w3):
    """Host-side fp8 term prep for one expert.

    x  [T, D] f32, w0 [F, D] f32, w3 [D, F] f32  ->  dict of device inputs.
    """
    f8 = ml_dtypes.float8_e4m3

    def q(a):
        return a.astype(f8)

    # GEMM1 moving operand: x split into main/residual terms
    a1 = q(x / GAM)
    a2 = q(x - a1.astype(np.float32))
    # [T, D] -> [P, NTBS, KC1, 2, TBS] with k = 256*kc + 128*i + p
    def x_layout(a):
        return np.ascontiguousarray(
            a.T.reshape(KC1, 2, P, NTBS, TBS).transpose(2, 3, 0, 1, 4))

    # GEMM1 stationary operand: scaled-transposed w0 terms
    w0t = (S * w0.T).astype(np.float32)           # [D, F]
    b1 = q(w0t)
    b2 = q(w0t * (1.0 + 1.0 / C) - b1.astype(np.float32) / C)
    # [D, F] -> [P, G, KC1, 2, FW]
    def w0_layout(a):
        return np.ascontiguousarray(
            a.reshape(KC1, 2, P, G, FW).transpose(2, 3, 0, 1, 4))

    # GEMM2 moving operand: scaled-transposed w3 terms
    w3t = (S * w3.T).astype(np.float32)           # [F, D]
    c1 = q(w3t)
    c2 = q(w3t * (1.0 + 1.0 / C) - c1.astype(np.float32) / C)
    # [F, D] -> [P, KC2, 2, D]
    def w3_layout(a):
        return np.ascontiguousarray(
            a.reshape(KC2, 2, P, D).transpose(2, 0, 1, 3))

    return {
        "xA1": x_layout(a1), "xA2": x_layout(a2),
        "w0B1": w0_layout(b1), "w0B2": w0_layout(b2),
        "w3C1": w3_layout(c1), "w3C2": w3_layout(c2),
    }


def kernel(**inputs):
    import os
    import sys
    if "/opt/trn_rl_repo" not in sys.path:
        sys.path.insert(0, "/opt/trn_rl_repo")
    from concourse import bass_utils

    output_tensor = np.asarray(inputs["output_tensor"], dtype=np.float32)  # [1, 8]
    x = np.asarray(inputs["inputs"], dtype=np.float32)   # [1, 8, 2048, 1024]
    w0 = np.asarray(inputs["w0"], dtype=np.float32)      # [8, 4096, 1024]
    w3 = np.asarray(inputs["w3"], dtype=np.float32)      # [8, 1024, 4096]

    from concurrent.futures import ThreadPoolExecutor
    with ThreadPoolExecutor(max_workers=N_CORES) as pool:
        in_maps = list(pool.map(
            lambda e: _prep_expert(x[0, e], w0[e], w3[e]), range(N_CORES)))

    nc = _get_nc()
    results = None
    if "fast_fn" in _cache:
        try:
            results = _cache["fast_fn"](in_maps)
        except Exception:
            results = None
    if results is None:
        try:
            results = bass_utils.run_bass_kernel_spmd(
                nc, in_maps, core_ids=list(range(N_CORES))).results
        except ModuleNotFoundError:
            # trace path requested via env but axon NTFF hook missing
            os.environ["BASS_NEVER_TRACE"] = "1"
            results = bass_utils.run_bass_kernel_spmd(
                nc, in_maps, core_ids=list(range(N_CORES))).results
        try:
            fast = _make_cached_fn(nc)
            fast(in_maps)  # warm: jit trace + XLA/NEFF compile happens here
            _cache["fast_fn"] = fast
        except Exception:
            pass
    out_full = np.stack([results[e]["out"] for e in range(N_CORES)])[None]

    # unpopular experts with zero gating activity produce zeros
    unpop = output_tensor[:, NUM_LOCAL:].sum(axis=0) != 0
    mask = np.concatenate([np.ones(NUM_LOCAL, dtype=bool), unpop])
    out_full = out_full * mask[None, :, None, None].astype(np.float32)
    return out_full.astype(np.float32)


# revision 27
# speedup vs baseline: 1.0028x; 1.0028x over previous
"""Trainium2 Bass kernel for nn_Experts (grouped MoE expert MLP).

Computes, for each of 8 experts e:
    h   = x_e @ w0_e.T          # [2048,1024] @ [1024,4096] -> [2048,4096]
    g   = gelu_exact(h)
    out = g @ w3_e.T            # [2048,4096] @ [4096,1024] -> [2048,1024]
then masks unpopular experts with zero gating activity (output_tensor).

Sharding: expert-parallel, 1 expert per NeuronCore across 8 cores (SPMD).

Compute strategy: error-compensated fp8 (e4m3) matmuls in DoubleRow perf
mode (K=256 per instruction, 0.5 PE cycles per output element -- 4x bf16
per fp8 pass). Plain fp8 quantization error (~5%) exceeds the accuracy
gate, so each GEMM runs TWO fp8 passes whose second ("residual") pass
cancels the first-order quantization error of BOTH operands:

    A1 = q(X/g),  A2 = q(X - A1)          (g = 1+c, c = 0.125)
    B1 = q(W),    B2 = q(W(1 + 1/c) - B1/c)
    X @ W  ~  A1@B1 + A2@B2               (error ~1e-2 absmax ratio)

The (1+c) factors cancel algebraically, so no rescale is needed beyond
the static weight scale S=64 folded into the GELU input scale / output
copy scale. Two fp8 passes at 4x = net 2x over the bf16 baseline.

x/w0/w3 terms are host-prepped; the GEMM2 stationary operand (gelu
output) is produced on device: ACT computes g_bf = gelu(h/S) (bf16) and
g1 = q(g_bf/g) (fp8 copy with scale), DVE computes gr = q(g_bf - g1).
"""

import numpy as np
import ml_dtypes

T = 2048      # tokens (capacity) per expert
D = 1024      # hidden
F = 4096      # ffn
P = 128       # partitions
TBS = 512     # token block
NTBS = T // TBS
KC1 = D // 256   # GEMM1 k-chunks (K=256 per DoubleRow matmul)
KC2 = F // 256   # GEMM2 k-chunks
G = 8            # w0 f-groups (DMA granularity)
FW = F // G      # f columns per group
NPAIR = F // 256  # gelu-tile pairs per block (16)
NTS = TBS // P    # t-subtiles per block (GEMM2 output partitions)
NUM_LOCAL = 4
N_CORES = 8

C = 0.125        # residual compensation ratio
GAM = 1.0 + C
S = 64.0         # static weight scale (keeps fp8(w) out of subnormals)

_cache = {}


def _build_nc(
    g_extra=1,
    h_bufs=3,
    o_ps_bufs=5,
    o_sb_bufs=3,
    x_bufs=2,
    gbf_bufs=10,
    warmup_mms=14,
):
    import sys
    if "/opt/trn_rl_repo" not in sys.path:
        sys.path.insert(0, "/opt/trn_rl_repo")
    import concourse.tile as tile
    import concourse.mybir as mybir
    from concourse import bacc

    bf16 = mybir.dt.bfloat16
    f32 = mybir.dt.float32
    fp8 = mybir.dt.float8e4
    AFT = mybir.ActivationFunctionType
    DR = mybir.MatmulPerfMode.DoubleRow
    SUB = mybir.AluOpType.subtract

    nc = bacc.Bacc(
        "TRN2",
        target_bir_lowering=False,
        debug=False,
        enable_asserts=True,
        num_devices=N_CORES,
    )

    # DRAM layouts: partition = contraction%128, plane dim of 2 for DoubleRow
    # (contraction index k = 256*kc + 128*i + p), grouped for long DMA runs.
    xA1 = nc.dram_tensor("xA1", [P, NTBS, KC1, 2, TBS], fp8, kind="ExternalInput").ap()
    xA2 = nc.dram_tensor("xA2", [P, NTBS, KC1, 2, TBS], fp8, kind="ExternalInput").ap()
    w0B1 = nc.dram_tensor("w0B1", [P, G, KC1, 2, FW], fp8, kind="ExternalInput").ap()
    w0B2 = nc.dram_tensor("w0B2", [P, G, KC1, 2, FW], fp8, kind="ExternalInput").ap()
    w3C1 = nc.dram_tensor("w3C1", [P, KC2, 2, D], fp8, kind="ExternalInput").ap()
    w3C2 = nc.dram_tensor("w3C2", [P, KC2, 2, D], fp8, kind="ExternalInput").ap()
    out = nc.dram_tensor("out", [T, D], f32, kind="ExternalOutput").ap()

    with tile.TileContext(nc) as tc:
        with (
            tc.tile_pool(name="weights", bufs=1) as wpool,
            tc.tile_pool(name="xin", bufs=x_bufs) as xpool,
            tc.tile_pool(name="gbf", bufs=gbf_bufs) as gbfpool,
            tc.tile_pool(name="g1p", bufs=NPAIR + g_extra) as g1pool,
            tc.tile_pool(name="grp", bufs=NPAIR + g_extra) as grpool,
            tc.tile_pool(name="ostage", bufs=o_sb_bufs) as opool,
            tc.tile_pool(name="hps", bufs=h_bufs, space="PSUM") as hpsum,
            tc.tile_pool(name="ops", bufs=o_ps_bufs, space="PSUM") as opsum,
        ):
            w0_sb = [wpool.tile([P, G, KC1, 2, FW], fp8, name=f"w0_sb{i}", tag=f"w0_sb{i}")
                     for i in range(2)]
            w3_sb = [wpool.tile([P, KC2, 2, D], fp8, name=f"w3_sb{i}", tag=f"w3_sb{i}")
                     for i in range(2)]

            x_tiles = {}

            def load_x(tb):
                xt = [xpool.tile([P, KC1, 2, TBS], fp8, name=f"x{i}_{tb}", tag=f"x{i}")
                      for i in range(2)]
                nc.sync.dma_start(xt[0][:], xA1[:, tb])
                nc.sync.dma_start(xt[1][:], xA2[:, tb])
                x_tiles[tb] = xt

            if warmup_mms:
                # ride out the PE cold-clock window on scratch matmuls while
                # the first DMAs are in flight
                with tc.tile_pool(name="warm", bufs=1) as warmpool:
                    wsrc = warmpool.tile([P, TBS], bf16, name="wsrc", tag="wsrc")
                    wps = opsum.tile([P, TBS], f32, name="wps", tag="o_ps")
                    nc.gpsimd.memset(wsrc[:], 0.0)
                    for i in range(warmup_mms):
                        nc.tensor.matmul(wps[:], wsrc[:, :P], wsrc[:],
                                         start=(i == 0), stop=(i == warmup_mms - 1))

            # critical prefix: w0 group 0, then x[tb0] (first matmul needs
            # both), then remaining w0 groups in consumption order, then w3
            # by k-chunk (phase B consumes kc2-outer), then the rest of x.
            nc.sync.dma_start(w0_sb[0][:, 0], w0B1[:, 0])
            nc.sync.dma_start(w0_sb[1][:, 0], w0B2[:, 0])
            load_x(0)
            w3_head = {3: 0, 6: 1}  # w3 chunks slipped into w0 stream slack
            for g in range(1, G):
                nc.sync.dma_start(w0_sb[0][:, g], w0B1[:, g])
                nc.sync.dma_start(w0_sb[1][:, g], w0B2[:, g])
                if g in w3_head:
                    kc2 = w3_head[g]
                    nc.sync.dma_start(w3_sb[0][:, kc2], w3C1[:, kc2])
                    nc.sync.dma_start(w3_sb[1][:, kc2], w3C2[:, kc2])
            for kc2 in range(len(w3_head), KC2):
                nc.sync.dma_start(w3_sb[0][:, kc2], w3C1[:, kc2])
                nc.sync.dma_start(w3_sb[1][:, kc2], w3C2[:, kc2])
            for tb in range(1, NTBS):
                load_x(tb)

            for tb in range(NTBS):
                xt = x_tiles.pop(tb)

                # phase A: GEMM1 (2-term fp8 DR) + GELU for all 32 f-tiles;
                # fp8 g1/gr pair tiles [P, 2, TBS] feed phase B as stationary.
                g1_t, gr_t = [], []
                for pair in range(NPAIR):
                    g1 = g1pool.tile([P, 2, TBS], fp8, name=f"g1_{tb}_{pair}", tag="g1")
                    gr = grpool.tile([P, 2, TBS], fp8, name=f"gr_{tb}_{pair}", tag="gr")
                    g1_t.append(g1)
                    gr_t.append(gr)
                    for i in range(2):
                        fc = 2 * pair + i
                        g, j = fc // (FW // P), fc % (FW // P)
                        h_ps = hpsum.tile([P, TBS], f32, name=f"h_{tb}_{fc}", tag="h_ps")
                        for term in range(2):
                            for kc in range(KC1):
                                nc.tensor.matmul(
                                    h_ps[:],
                                    w0_sb[term][:, g, kc, :, j * P:(j + 1) * P],
                                    xt[term][:, kc],
                                    start=(term == 0 and kc == 0),
                                    stop=(term == 1 and kc == KC1 - 1),
                                    perf_mode=DR,
                                )
                        g_bf = gbfpool.tile([P, TBS], bf16, name=f"gb_{tb}_{fc}", tag="g_bf")
                        # gelu alone on ACT (gates h_ps release; must outpace
                        # PE's 853ns/f-tile); g1/gr on DVE (delay-tolerant)
                        nc.scalar.activation(g_bf[:], h_ps[:], AFT.Gelu, scale=1.0 / S)
                        nc.vector.tensor_scalar_mul(g1[:, i], g_bf[:], 1.0 / GAM)
                        nc.vector.tensor_tensor(gr[:, i], g_bf[:], g1[:, i], SUB)

                # phase B: GEMM2 over the 8 [128t x 512d] psum out tiles in
                # two kc2-outer groups of 5 then 3: kc2-outer ordering lets
                # w3 stream in k-chunk order (block 0) and tolerates DVE
                # g1/gr lag; the bigger first group keeps PE fed while w3
                # still streams, the smaller second group shrinks the
                # post-delivery tail. The final group of the last block runs
                # tile-sequential so copies/DMA-out pipeline behind the
                # matmuls instead of serializing after the last one.
                tiles = [(ts, dc) for ts in range(NTS) for dc in range(2)]
                # block 0: bigger first group = more PE work per streamed-in
                # w3 k-chunk (higher tolerance to the w3 DMA stream)
                split = 5 if tb == 0 else 4
                groups = [tiles[:split], tiles[split:]]

                def emit_mm(o_t, ts, dc, kc2):
                    for term in range(2):
                        gt = (g1_t, gr_t)[term][kc2]
                        nc.tensor.matmul(
                            o_t[:],
                            gt[:, :, ts * P:(ts + 1) * P],
                            w3_sb[term][:, kc2, :, dc * TBS:(dc + 1) * TBS],
                            start=(kc2 == 0 and term == 0),
                            stop=(kc2 == KC2 - 1 and term == 1),
                            perf_mode=DR,
                        )

                def emit_copy(o_t, ts, dc, splits=1, eng=None):
                    # group1 copies go to DVE (drained by end of phase B) so
                    # ACT's queue holds only gelus at the next phase-A start
                    # — gelu latency gates h_ps reuse
                    o_sb = opool.tile([P, TBS], f32, name=f"os_{tb}_{ts}_{dc}",
                                      tag="o_sb")
                    w = TBS // splits
                    for s in range(splits):
                        if eng is nc.scalar:
                            nc.scalar.mul(o_sb[:, s * w:(s + 1) * w],
                                          o_t[:, s * w:(s + 1) * w], 1.0 / S)
                        else:
                            nc.vector.tensor_scalar_mul(
                                o_sb[:, s * w:(s + 1) * w],
                                o_t[:, s * w:(s + 1) * w], 1.0 / S)
                        nc.sync.dma_start(
                            out[tb * TBS + ts * P: tb * TBS + (ts + 1) * P,
                                dc * TBS + s * w: dc * TBS + (s + 1) * w],
                            o_sb[:, s * w:(s + 1) * w],
                        )

                for gi, group in enumerate(groups):
                    o_ps = {td: opsum.tile([P, TBS], f32,
                                           name=f"o_{tb}_{td[0]}_{td[1]}", tag="o_ps")
                            for td in group}
                    if tb == NTBS - 1 and gi == 1:
                        for i, (ts, dc) in enumerate(group):
                            last = i == len(group) - 1
                            for kc2 in range(KC2):
                                emit_mm(o_ps[(ts, dc)], ts, dc, kc2)
                            emit_copy(o_ps[(ts, dc)], ts, dc,
                                      splits=2 if last else 1,
                                      eng=nc.scalar if last else None)
                    else:
                        for kc2 in range(KC2):
                            for ts, dc in group:
                                emit_mm(o_ps[(ts, dc)], ts, dc, kc2)
                        for ts, dc in group:
                            emit_copy(o_ps[(ts, dc)], ts, dc, eng=nc.scalar)

    nc.compile()
    return nc


def _get_nc():
    if "nc" not in _cache:
        _cache["nc"] = _build_nc()
    return _cache["nc"]


def _make_cached_fn(nc):
    """Build a reusable jitted 8-core executable around bass2jax's bass_exec
    primitive (the same lowering run_bass_kernel_spmd uses under axon), so
    repeat kernel() calls skip retrace/relower."""
    import jax
    import numpy as np
    from jax.sharding import Mesh, PartitionSpec
    try:
        from jax.experimental.shard_map import shard_map
    except ImportError:
        from jax.shard_map import shard_map
    import concourse.mybir as mybir
    from concourse.bass2jax import (_bass_exec_p, install_neuronx_cc_hook,
                                    partition_id_tensor)

    install_neuronx_cc_hook()
    partition_name = nc.partition_id_tensor.name if nc.partition_id_tensor else None
    in_names, out_names, out_avals, zero_shapes = [], [], [], []
    for alloc in nc.m.functions[0].allocations:
        if not isinstance(alloc, mybir.MemoryLocationSet):
            continue
        name = alloc.memorylocations[0].name
        if alloc.kind == "ExternalInput":
            if name != partition_name:
                in_names.append(name)
        elif alloc.kind == "ExternalOutput":
            out_names.append(name)
            shape = tuple(alloc.tensor_shape)
            dtype = mybir.dt.np(alloc.dtype)
            out_avals.append(jax.core.ShapedArray(shape, dtype))
            zero_shapes.append((shape, dtype))
    n_params = len(in_names)
    all_in_names = list(in_names) + list(out_names)
    if partition_name is not None:
        all_in_names.append(partition_name)

    def _body(*args):
        ins = list(args[:n_params])
        outs = list(args[n_params:])
        extra = [partition_id_tensor()] if partition_name is not None else []
        return tuple(_bass_exec_p.bind(
            *ins, *outs, *extra,
            out_avals=tuple(out_avals),
            in_names=tuple(all_in_names),
            out_names=tuple(out_names),
            lowering_input_output_aliases=(),
            sim_require_finite=True,
            sim_require_nnan=True,
            nc=nc,
        ))

    devices = jax.devices()[:N_CORES]
    mesh = Mesh(np.asarray(devices), ("core",))
    fn = jax.jit(
        shard_map(_body, mesh=mesh,
                  in_specs=(PartitionSpec("core"),) * (n_params + len(out_names)),
                  out_specs=(PartitionSpec("core"),) * len(out_names),
                  check_rep=False),
        keep_unused=True)

    def run(in_maps):
        concat_in = [np.concatenate([np.asarray(m[n]) for m in in_maps], axis=0)
                     for n in in_names]
        concat_zeros = [np.zeros((N_CORES * s[0], *s[1:]), dt)
                        for s, dt in zero_shapes]
        outs = fn(*concat_in, *concat_zeros)
        return [
            {name: np.asarray(outs[i]).reshape(N_CORES, *out_avals[i].shape)[c]
             for i, name in enumerate(out_names)}
            for c in range(N_CORES)
        ]

    return run


def _prep_expert(x, w0, w3):
    """Host-side fp8 term prep for one expert.

    x  [T, D] f32, w0 [F, D] f32, w3 [D, F] f32  ->  dict of device inputs.
    """
    f8 = ml_dtypes.float8_e4m3

    def q(a):
        return a.astype(f8)

    # GEMM1 moving operand: x split into main/residual terms
    a1 = q(x / GAM)
    a2 = q(x - a1.astype(np.float32))
    # [T, D] -> [P, NTBS, KC1, 2, TBS] with k = 256*kc + 128*i + p
    def x_layout(a):
        return np.ascontiguousarray(
            a.T.reshape(KC1, 2, P, NTBS, TBS).transpose(2, 3, 0, 1, 4))

    # GEMM1 stationary operand: scaled-transposed w0 terms
    w0t = (S * w0.T).astype(np.float32)           # [D, F]
    b1 = q(w0t)
    b2 = q(w0t * (1.0 + 1.0 / C) - b1.astype(np.float32) / C)
    # [D, F] -> [P, G, KC1, 2, FW]
    def w0_layout(a):
        return np.ascontiguousarray(
            a.reshape(KC1, 2, P, G, FW).transpose(2, 3, 0, 1, 4))

    # GEMM2 moving operand: scaled-transposed w3 terms
    w3t = (S * w3.T).astype(np.float32)           # [F, D]
    c1 = q(w3t)
    c2 = q(w3t * (1.0 + 1.0 / C) - c1.astype(np.float32) / C)
    # [F, D] -> [P, KC2, 2, D]
    def w3_layout(a):
        return np.ascontiguousarray(
            a.reshape(KC2, 2, P, D).transpose(2, 0, 1, 3))

    return {
        "xA1": x_layout(a1), "xA2": x_layout(a2),
        "w0B1": w0_layout(b1), "w0B2": w0_layout(b2),
        "w3C1": w3_layout(c1), "w3C2": w3_layout(c2),
    }


def kernel(**inputs):
    import os
    import sys
    if "/opt/trn_rl_repo" not in sys.path:
        sys.path.insert(0, "/opt/trn_rl_repo")
    from concourse import bass_utils

    output_tensor = np.asarray(inputs["output_tensor"], dtype=np.float32)  # [1, 8]
    x = np.asarray(inputs["inputs"], dtype=np.float32)   # [1, 8, 2048, 1024]
    w0 = np.asarray(inputs["w0"], dtype=np.float32)      # [8, 4096, 1024]
    w3 = np.asarray(inputs["w3"], dtype=np.float32)      # [8, 1024, 4096]

    from concurrent.futures import ThreadPoolExecutor
    with ThreadPoolExecutor(max_workers=N_CORES) as pool:
        in_maps = list(pool.map(
            lambda e: _prep_expert(x[0, e], w0[e], w3[e]), range(N_CORES)))

    nc = _get_nc()
    results = None
    if "fast_fn" in _cache:
        try:
            results = _cache["fast_fn"](in_maps)
        except Exception:
            results = None
    if results is None:
        try:
            results = bass_utils.run_bass_kernel_spmd(
                nc, in_maps, core_ids=list(range(N_CORES))).results
        except ModuleNotFoundError:
            # trace path requested via env but axon NTFF hook missing
            os.environ["BASS_NEVER_TRACE"] = "1"
            results = bass_utils.run_bass_kernel_spmd(
                nc, in_maps, core_ids=list(range(N_CORES))).results
        try:
            fast = _make_cached_fn(nc)
            fast(in_maps)  # warm: jit trace + XLA/NEFF compile happens here
            _cache["fast_fn"] = fast
        except Exception:
            pass
    out_full = np.stack([results[e]["out"] for e in range(N_CORES)])[None]

    # unpopular experts with zero gating activity produce zeros
    unpop = output_tensor[:, NUM_LOCAL:].sum(axis=0) != 0
    mask = np.concatenate([np.ones(NUM_LOCAL, dtype=bool), unpop])
    out_full = out_full * mask[None, :, None, None].astype(np.float32)
    return out_full.astype(np.float32)
